# revision 40
# baseline (speedup 1.0000x reference)
"""Grouped MLP (8-expert SwiGLU) Trainium2 Bass kernel, v3 (682.5us).

Sharding: expert-parallel, one group per NeuronCore (8 cores).
Token t belongs to group t % 8, so core n gets x[n::8] (4096 tokens),
its expert's gate/up/down weights, and produces out[n::8].

v2 changes over the fp32r baseline (726.3us):
- All matmuls in bf16: the PE streams bf16 moving data at 1.0 cycle/row
  (512-row MM spacing measured 215.8ns) while fp32r pays a 17/16 row tax
  (226.7ns).  NOTE: float16 is NOT usable here - kernels with a large
  fp16 matmul count get statically downclocked to 2.0GHz chip-wide
  (259ns/MM); bf16 keeps the full 2.4GHz.  Accumulation stays fp32 in
  PSUM; end-to-end error vs the fp32 reference is 4.1e-3 (gate 2e-2).
  fp8 e4m3 (2x PE rate) measures 6e-2 end-to-end - fails the gate;
  split-residual fp8 passes (2e-3) but needs 3 MMs = 1.5x bf16.  Dead
  ends both.
- Host pre-packs every tensor into the exact SBUF tile layout, so each
  DMA is one fully contiguous read (2KB+ per partition row, line rate).
- Dual HWDGE rings (sync + scalar are the ONLY hwdge queues on TRN2):
  gate weights + output writes on sync, up weights + activations
  (+half of wd) on scalar.  At kernel start all 8 cores burst-saturate
  HBM (~76GB/s per ring); BALANCED bytes across the rings is what
  matters - an all-on-one-ring xt0 measured 1.5us slower, and >512
  descriptors per DMA hits ring-full backpressure.
- Cross-block prefetch: xt and the first two weight chunk-pairs of
  block tb+1 issue before block tb's down-projection loop, so the PE
  never waits at a block boundary.

v3 changes (684.8 -> 682.5us), all driven by the HAM duty model: the
chip grants ONE contiguous full-duty (k=8/8) window of exactly 672.4us
starting when sustained PE activity begins (grant lag 2.5-5.7us after
the first MM, run-to-run thermal noise); outside it everything runs at
k=4/8 (half clock).  exec ~= grant + 672.4 + 2*(work left after the
window).  Consequences engineered here:
- SiLU act-table preload: a dummy ACTIVATE right after the startup
  scalar-ring DMAs makes Tile emit the 1.28us ACT_TABLE_LOAD at ~+10us
  (otherwise it lands at +16.9us and stalls the PE 2.1us waiting on the
  PSUM bank the first real ACTIVATE frees).
- PSUM banks 3/3/2 (gate/up/outp): with only 2, whichever of gate/up
  has 2 banks stalls ~2-3us at the ramp exit behind the scalar-queue
  epilogue cadence (ACTIVATE -> TENSOR_TENSOR frees the bank).
- Warm-up delayed to ~+9.5us (two dummy vector memsets ahead of the
  scratch memset) + 10 MMs: biases the HAM grant toward the real-chain
  start (~+13.2us) so no full-duty budget is wasted on the startup DMA
  wait.  Real chains then run gap-free (measured 0ns of PE gaps >60ns).
- Tail keep-warm: an 11-MM dummy chain after the last real MM holds
  k=8/8 through the output drain (activity + ~4.8us grace covers to the
  budget end); without it the final copy+DMA+barrier ran at half clock.
- Last output tile computed as 256+224+32-wide chains with the final
  DMAs hand-placed on the emptier ring: past the budget end every ns of
  drain costs 2ns; the ~2.4us DMA completion latency dominates the
  post-MM tail (last-MM -> final-DMA-complete ~3.3us, then a ~6.3us
  framework barrier epilogue that cannot be shortened from kernel code).
"""

import sys

if "/opt/trn_rl_repo" not in sys.path:
    sys.path.insert(0, "/opt/trn_rl_repo")

import ml_dtypes
import numpy as np

import concourse.bass as bass  # noqa: F401  (registers bass machinery)
import concourse.tile as tile
from concourse import bacc, mybir
from concourse.bass_utils import run_bass_kernel_spmd

P = 128
T = 4096   # tokens per core (per group)
K = 1024   # d_in
H = 2048   # d_hid
O = 1024   # d_out
N_CORES = 8

F16 = mybir.dt.bfloat16
F32 = mybir.dt.float32

# Tiling knobs
TB = 1024           # token block
WCH = 128           # gate/up weight chunk width along hidden dim
MMF = 512           # matmul moving free dim (one fp32 PSUM bank)

KO = K // P         # 8  k-subtiles
HO = H // P         # 16 h-subtiles
NTB = T // TB       # token blocks
NT = TB // MMF      # 512-wide t-tiles per token block
NWC = H // WCH      # weight chunks per block
NO = O // MMF       # 512-wide o-tiles

_CACHED_NC = None


def _build_nc():
    from contextlib import ExitStack

    nc = bacc.Bacc(None, target_bir_lowering=False)
    # Host-packed layouts (all contiguous in the order the DMAs read them):
    #   xt:  [NTB][P][KO][TB]    x transposed + k-tiled, per token block
    #   wg/wu: [NWC][P][KO][WCH] weight chunks, k-tiled
    #   wd:  [HO][P][O]          down weights, h-tiled
    xt = nc.dram_tensor("xt", [NTB, P, KO, TB], F16, kind="ExternalInput")
    wg = nc.dram_tensor("wg", [NWC, P, KO, WCH], F16, kind="ExternalInput")
    wu = nc.dram_tensor("wu", [NWC, P, KO, WCH], F16, kind="ExternalInput")
    wd = nc.dram_tensor("wd", [HO, P, O], F16, kind="ExternalInput")
    out = nc.dram_tensor("out", [T, O], F32, kind="ExternalOutput")

    silu_fn = mybir.ActivationFunctionType.Silu

    with tile.TileContext(nc) as tc, ExitStack() as ctx:
        const = ctx.enter_context(tc.tile_pool(name="const", bufs=1))
        xpool = ctx.enter_context(tc.tile_pool(name="xpool", bufs=2))
        wpool = ctx.enter_context(tc.tile_pool(name="wpool", bufs=4))
        hpool = ctx.enter_context(tc.tile_pool(name="hpool", bufs=1))
        spool = ctx.enter_context(tc.tile_pool(name="spool", bufs=2))
        opool = ctx.enter_context(tc.tile_pool(name="opool", bufs=3))
        # PSUM budget is 8 banks of [128,512]f32: gate 3 + up 3 + outp 2.
        # Both gate and up chains stall at the ramp exit when they only
        # have 2 banks (first MM waits the ACTIVATE / TENSOR_TENSOR that
        # frees the bank, and the scalar-queue epilogue cadence lags the
        # PE during the startup DMA burst).  The down phase tolerates
        # outp=2: copy(k) completes ~4.2us after chain k starts, chain
        # k+2 starts 6.9us after.  The warm-up tile shares the outp
        # tag/rotation (its chain is long done before the first
        # down-projection needs the bank).
        ps12 = ctx.enter_context(tc.tile_pool(name="ps12", bufs=3, space="PSUM"))
        ps3 = ctx.enter_context(tc.tile_pool(name="ps3", bufs=2, space="PSUM"))

        # --- PE warm-up: one dummy chain on zeroed scratch.  The HAM
        # grants the fixed 672.4us full-duty budget ~2.5-5.7us after PE
        # activity starts; an early grant wastes budget on the startup
        # DMA wait (real chains only start at ~+13.4us once the first xt
        # quadrant lands) and pushes the end-of-kernel barrier past the
        # budget into half-duty.  Two dummy memsets ahead of scratch on
        # the vector queue delay the first warm-up MM to ~+9.5us, biasing
        # the grant toward the real-chain start; 10 MMs then bridge
        # activity to ~+13.4us.
        delay0 = const.tile([P, 1024], F16)
        delay1 = const.tile([P, 1024], F16)
        nc.vector.memset(delay0[:], 0)
        nc.vector.memset(delay1[:], 0)
        scratch = const.tile([P, MMF], F16)
        nc.vector.memset(scratch[:], 0)
        # 10 MMs: at half duty (late grant) the chain ends ~+14.6us, at
        # full ~+13.3us; xt quadrant arrival is itself noisy (+13.4-16us
        # across runs), and the longer bridge measured best on average.
        warm_ps = ps3.tile([P, MMF], F32, tag="outp")
        for i in range(10):
            nc.tensor.matmul(
                warm_ps[:],
                scratch[:, 0:P],
                scratch[:],
                start=(i == 0),
                stop=(i == 9),
            )

        # Down-projection weights resident for the whole kernel.
        wd_sb = const.tile([P, HO, O], F16)

        # Persistent tile handles across the tb loop (allocated per tb).
        xt_tiles = {}

        def issue_xt(tb):
            xt_sb = xpool.tile([P, KO, TB], F16, tag="xt", name=f"xt{tb}")
            nc.scalar.dma_start(xt_sb[:], xt[tb])
            xt_tiles[tb] = xt_sb

        wg_tiles = {}
        wu_tiles = {}

        def issue_wg(tb, wc):
            wg_sb = wpool.tile([P, KO, WCH], F16, tag="wg", name=f"wg{tb}_{wc}")
            nc.sync.dma_start(wg_sb[:], wg[wc])
            wg_tiles[(tb, wc)] = wg_sb

        def issue_wu(tb, wc):
            wu_sb = wpool.tile([P, KO, WCH], F16, tag="wu", name=f"wu{tb}_{wc}")
            nc.scalar.dma_start(wu_sb[:], wu[wc])
            wu_tiles[(tb, wc)] = wu_sb

        # tb0 startup: xt block 0 in quadrants across both rings; the
        # first 256-wide chains need only quadrant 0.  At kernel start all
        # 8 cores burst-saturate HBM and each ring only sustains
        # ~75-90GB/s — balancing bytes across the two rings is what
        # matters (an all-on-one-ring xt0 measured 1.5us slower), and the
        # half-quadrant pieces keep each DMA at 512 descriptors (a
        # full-depth 1024-desc quadrant hit ring-full backpressure).
        issue_wg(0, 0)      # sync ring:   wg0 (first gate chain's weights)
        issue_wu(0, 0)      # scalar ring: wu0 (first up chain's weights)
        # xt0 pieces match the ramp chain widths (128,128,256,256,256
        # tokens): the first complete chain then needs only ~320KB/ring
        # (wg0/wu0 + 64KB of xt per ring) instead of 512KB, starting the
        # PE ~2us earlier at the throttled pre-grant DMA rate.
        # Finer xt0 pieces (128-token ramp prefix) were tried three ways:
        # the first chain starts ~2us earlier, but pre-grant the two
        # rings have zero slack — every placement of wg1/wu1 relative to
        # the xt tail produced a new 1-2us ramp stall (measured 683.1,
        # 685.4, 683.3 vs 682.5 for this layout).  The simple 256-token
        # quadrant split with weights bracketing it is the stable optimum.
        xt0_sb = xpool.tile([P, KO, TB], F16, tag="xt", name="xt0")
        for q in range(0, TB, 256):
            nc.sync.dma_start(xt0_sb[:, 0:4, q : q + 256], xt[0, :, 0:4, q : q + 256])
            nc.scalar.dma_start(xt0_sb[:, 4:8, q : q + 256], xt[0, :, 4:8, q : q + 256])
        xt_tiles[0] = xt0_sb
        issue_wg(0, 1)      # keep chunk 1 weights ahead of the wd weave
        issue_wu(0, 1)

        # SiLU act-table preload: Tile emits ACT_TABLE_LOAD right before
        # the first ACTIVATE in the Scalar queue's program order.  Without
        # this dummy, the 1.28us table load lands just before the first
        # real SiLU (~+16.9us) and stalls the PE 2.3us waiting on the
        # PSUM bank that ACTIVATE frees.  Here it issues after the
        # startup-critical scalar-ring DMAs (wu0/xt halves/wu1) and
        # completes ~+11.7us, before the first real ACTIVATE (~+13us).
        scratch32 = const.tile([P, 8], F32)
        nc.vector.memset(scratch32[:], 0)
        warm_act = const.tile([P, 8], F32)
        nc.scalar.activation(warm_act[:], scratch32[:], silu_fn)

        def gateup(wg_sb, wu_sb, h, tsl, xt_sb, hid_sb):
            mmf = tsl.stop - tsl.start
            gate_ps = ps12.tile([P, mmf], F32, tag="gate")
            for ko in range(KO):
                nc.tensor.matmul(
                    gate_ps[:],
                    wg_sb[:, ko, :],
                    xt_sb[:, ko, tsl],
                    start=(ko == 0),
                    stop=(ko == KO - 1),
                )
            up_ps = ps12.tile([P, mmf], F32, tag="up")
            for ko in range(KO):
                nc.tensor.matmul(
                    up_ps[:],
                    wu_sb[:, ko, :],
                    xt_sb[:, ko, tsl],
                    start=(ko == 0),
                    stop=(ko == KO - 1),
                )
            silu_sb = spool.tile([P, mmf], F32, tag="silu")
            nc.scalar.activation(silu_sb[:], gate_ps[:], silu_fn)
            nc.vector.tensor_mul(hid_sb[:, h, tsl], silu_sb[:], up_ps[:])

        for tb in range(NTB):
            xt_sb = xt_tiles.pop(tb)
            hid_sb = hpool.tile([P, HO, TB], F16, tag="hid")

            if tb == 0:
                # Startup ramp: chunk 0 in 256-wide t-tiles (each needs
                # only one xt quadrant), then chunk 1 at full width — the
                # PE starts on partial xt while the rest streams in.
                wg0_sb = wg_tiles.pop((0, 0))
                wu0_sb = wu_tiles.pop((0, 0))
                wg1_sb = wg_tiles.pop((0, 1))
                wu1_sb = wu_tiles.pop((0, 1))
                for wpair, h, lo, hi in (
                    ((wg0_sb, wu0_sb), 0, 0, 256),
                    ((wg0_sb, wu0_sb), 0, 256, 512),
                    ((wg0_sb, wu0_sb), 0, 512, 768),
                    ((wg0_sb, wu0_sb), 0, 768, 1024),
                    ((wg1_sb, wu1_sb), 1, 0, 512),
                    ((wg1_sb, wu1_sb), 1, 512, 1024),
                ):
                    gateup(wpair[0], wpair[1], h, slice(lo, hi), xt_sb, hid_sb)

            for wc in range(2 if tb == 0 else 0, NWC):
                if (tb, wc) not in wg_tiles:
                    issue_wg(tb, wc)
                if (tb, wc) not in wu_tiles:
                    issue_wu(tb, wc)
                wg_sb = wg_tiles.pop((tb, wc))
                wu_sb = wu_tiles.pop((tb, wc))
                if tb == 0 and 8 <= wc < 16:
                    # Weave the resident down-projection weights between
                    # block-0 chunks (two per chunk, one per ring), in the
                    # back half so the early weight stream is never delayed;
                    # still complete long before the down phase reads them.
                    ho2 = (wc - 8) * 2
                    nc.sync.dma_start(wd_sb[:, ho2, :], wd[ho2])
                    nc.scalar.dma_start(wd_sb[:, ho2 + 1, :], wd[ho2 + 1])

                for th in range(NT):
                    gateup(wg_sb, wu_sb, wc, slice(th * MMF, (th + 1) * MMF),
                           xt_sb, hid_sb)

            # Prefetch next block's activations + first chunk-pairs before
            # the down loop, so the PE never waits at the block boundary.
            if tb + 1 < NTB:
                issue_xt(tb + 1)
                issue_wg(tb + 1, 0)
                issue_wg(tb + 1, 1)
                issue_wu(tb + 1, 0)
                issue_wu(tb + 1, 1)

            # Down projection for this token block.  The very last output
            # tile of the kernel is computed as two 256-wide chains so the
            # tail (final copy + DMA after the last matmul) is halved.
            # Output DMAs alternate between the sync and scalar rings so
            # the end-of-kernel drain is not serialized on one ring: the
            # last few writes land on rings with empty queues.
            for ti in range(TB // P):
                for oi in range(NO):
                    last = tb == NTB - 1 and ti == TB // P - 1 and oi == NO - 1
                    # The kernel's very last output tile is computed as
                    # 256+224+32-wide chains: the final copy+DMA after the
                    # last matmul gates the barrier-epilogue start, and
                    # past the duty budget every ns there costs 2ns — so
                    # the last piece is as small as possible (16KB; the
                    # ~2.3us DMA completion latency dominates).
                    osls = (
                        [slice(oi * MMF, oi * MMF + 256),
                         slice(oi * MMF + 256, oi * MMF + 480),
                         slice(oi * MMF + 480, (oi + 1) * MMF)]
                        if last
                        else [slice(oi * MMF, (oi + 1) * MMF)]
                    )
                    for si, osl in enumerate(osls):
                        width = osl.stop - osl.start
                        out_ps = ps3.tile([P, width], F32, tag="outp")
                        for ho in range(HO):
                            nc.tensor.matmul(
                                out_ps[:],
                                hid_sb[:, ho, ti * P : (ti + 1) * P],
                                wd_sb[:, ho, osl],
                                start=(ho == 0),
                                stop=(ho == HO - 1),
                            )
                        ob = opool.tile([P, width], F32, tag="ob")
                        nc.vector.tensor_copy(ob[:], out_ps[:])
                        # Blocks 0..NTB-2: keep every out DMA on the sync
                        # ring (the scalar ring is draining the 2MB xt
                        # prefetch; queueing behind it would stall opool).
                        # Last block: alternate rings, and hand-place the
                        # final pieces so the kernel's very last DMA (the
                        # 64-wide piece) rides a ring that drained ~1us
                        # earlier — its completion gates the epilogue
                        # barrier, and past the 672us full-duty budget
                        # every ns of drain costs 2ns.
                        if tb == NTB - 1:
                            if last:
                                ring = nc.scalar if si < 2 else nc.sync
                            elif ti == TB // P - 1 and oi == 0:
                                ring = nc.sync
                            elif (ti + oi + si) % 2 == 1:
                                ring = nc.scalar
                            else:
                                ring = nc.sync
                        else:
                            ring = nc.sync
                        ring.dma_start(
                            out[tb * TB + ti * P : tb * TB + (ti + 1) * P, osl],
                            ob[:],
                        )

        # --- Tail keep-warm: when PE activity stops, the HAM drops the
        # chip to 4/8 duty after a ~4.8us grace window, so the final
        # copy+DMA drain and the framework's barrier epilogue run at half
        # clock (measured 7.5-8.9us at k=4).  A ~2.4us dummy chain after
        # the last real matmul keeps the duty at 8/8 through the drain;
        # it depends only on scratch + the long-freed gate rotation, so
        # it issues immediately with no wait.  11 MMs end right when the
        # final out DMA completes (~last-MM + 2.4us) — any longer and the
        # Tensor engine, not the DMA, gates the barrier-epilogue entry.
        tail_ps = ps12.tile([P, MMF], F32, tag="gate")
        for i in range(11):
            nc.tensor.matmul(
                tail_ps[:],
                scratch[:, 0:P],
                scratch[:],
                start=(i == 0),
                stop=(i == 10),
            )

    nc.compile()
    return nc


def _get_nc():
    global _CACHED_NC
    if _CACHED_NC is None:
        _CACHED_NC = _build_nc()
    return _CACHED_NC


def _pack_weights(w):
    """[K, H] -> [NWC, P, KO, WCH] bf16, contiguous."""
    # w[ko*P + p, wc*WCH + h] -> wp[wc, p, ko, h]
    w4 = w.reshape(KO, P, NWC, WCH).transpose(2, 1, 0, 3)
    return np.ascontiguousarray(w4).astype(ml_dtypes.bfloat16)


def _pack_xt(xg):
    """[T, K] tokens-of-group -> [NTB, P, KO, TB] bf16, contiguous."""
    # xg[tb*TB + t, ko*P + p] -> xp[tb, p, ko, t]
    x4 = xg.reshape(NTB, TB, KO, P).transpose(0, 3, 2, 1)
    return np.ascontiguousarray(x4).astype(ml_dtypes.bfloat16)


def _pack_wd(w):
    """[H, O] -> [HO, P, O] bf16, contiguous."""
    w3 = w.reshape(HO, P, O)
    return np.ascontiguousarray(w3).astype(ml_dtypes.bfloat16)


def _make_in_maps(x, gate_weight, up_weight, down_weight, n):
    in_maps = []
    for g in range(n):
        in_maps.append(
            {
                "xt": _pack_xt(x[g::n]),
                "wg": _pack_weights(gate_weight[g]),
                "wu": _pack_weights(up_weight[g]),
                "wd": _pack_wd(down_weight[g]),
            }
        )
    return in_maps


def _run_spmd(in_maps, **kwargs):
    nc = _get_nc()
    return run_bass_kernel_spmd(nc, in_maps, core_ids=list(range(N_CORES)), **kwargs)


def kernel(x, gate_weight, up_weight, down_weight, num_groups=8):
    n = int(num_groups)
    x = np.asarray(x, dtype=np.float32)
    gate_weight = np.asarray(gate_weight, dtype=np.float32)
    up_weight = np.asarray(up_weight, dtype=np.float32)
    down_weight = np.asarray(down_weight, dtype=np.float32)

    assert n == N_CORES, f"expected {N_CORES} groups, got {n}"
    assert x.shape == (T * N_CORES, K), x.shape
    assert gate_weight.shape == (n, K, H), gate_weight.shape
    assert up_weight.shape == (n, K, H), up_weight.shape
    assert down_weight.shape == (n, H, O), down_weight.shape

    in_maps = _make_in_maps(x, gate_weight, up_weight, down_weight, n)
    res = _run_spmd(in_maps)

    out = np.empty((x.shape[0], O), dtype=np.float32)
    for g in range(n):
        out[g::n] = res.results[g]["out"]
    return out

